# revision 14
# baseline (speedup 1.0000x reference)
"""NeuralCDE Bass kernel for Trainium2 (8 NeuronCores, data parallel).

Computes the reference NeuralCDE: cubic-spline-controlled ODE integrated with
torchdiffeq-style rk4 (3/8 rule) over 254 fixed steps, MLP vector field
(32 -> 128 -> 128 -> 32x8 with tanh), initial MLP on device, readout MLP on
host.

Layout (per core, batch BC=2048, two anti-phased groups of GN=1024):
  - batch split into 2 groups of 1024, each 2 subchunks of 512; the two
    groups' pipelines are emitted anti-phased (half-eval stage offset) so
    one group's PE matmuls overlap the other group's ACT/DVE stages.
  - activations are feature-major: z packed as (64, 512) tiles with row
    32*q + h (q = local subchunk, h = hidden dim), batch along free dim.
  - MLP: mm1 fp32r (K=32 per q via tile_position rows 0/32 -- rows 64/96
    crash the device, so subchunk count stays at 2), mm2/mm3 bf16, tanh
    on ACT with fused bias (bf16 out).
  - spline derivative: rep = [R; f R; f^2 R]^T @ coeff-slab on PE into a
    single (128,1024) psum tile (R replicates channel c to rows j%8==c).
  - P = tanh(F) * rep computed by DVE into an fp8e4 (128, 2, 1024) tile
    (plane i = F_i row block); channel sum k = S^T P via one fp8
    DoubleRow matmul per subchunk (K=2x128 planes in a single pass) into
    kacc (32, 1024) -- 2-4x cheaper on PE than the bf16 2-pass sum.
  - rk4 z-updates via scalar_tensor_tensor identities consuming each k_i
    immediately:
       z2 = z + (dt/3) k1
       z3 = dt*k2 + (2z - z2)
       z4 = dt*k3 + (2*z2 - z3)
       z5 = ((dt*k4 + (3*z4 + (6*z3 - z)))) / 8
    k-consuming STTs read psum and stay on DVE; the SBUF-only ops
    (tmp/t3/t4/znew) can be offloaded to the idle GpSimd engine
    (NCDE_GPS=1) to unload DVE.
"""

import os
import sys
import time

sys.path.insert(0, "/opt/trn_rl_repo")

import numpy as np

import concourse.bacc as bacc
import concourse.bass as bass
from concourse import bass_utils, mybir, tile

F32 = mybir.dt.float32
F32R = mybir.dt.float32r
BF16 = mybir.dt.bfloat16
FP8 = mybir.dt.float8e4
AF = mybir.ActivationFunctionType
OP = mybir.AluOpType
DR = mybir.MatmulPerfMode.DoubleRow

KSUM_MODE = os.environ.get("NCDE_KSUM", "dr")   # dr | fp8acc
GPS = bool(int(os.environ.get("NCDE_GPS", "0")))

CORES = 8
B = 16384
BC = B // CORES          # 2048 batch per core
SUB = 512                # subchunk batch
GN = 2 * SUB             # 1024 batch per group
L = 128                  # knots
NPIECE = L - 1           # 127
STEPS = 2 * (L - 1)      # 254
HID = 32
C = 8
DT = 0.5

_BUILD_CACHE = {}


def _schedule(num_steps):
    """Per (step, eval) -> (piece, frac_index); frac = fi/6."""
    sched = []
    for s in range(num_steps):
        evs = []
        for e in range(4):
            T = 3 * s + e
            idx = min(T // 6, NPIECE - 1)
            fi = T - 6 * idx
            evs.append((idx, fi))
        sched.append(evs)
    return sched


def _build(num_steps=STEPS, n_pieces=NPIECE, n_knots=L):
    key = (num_steps, n_pieces, n_knots)
    if key in _BUILD_CACHE:
        return _BUILD_CACHE[key]

    t_start = time.time()
    nc = bacc.Bacc("TRN2", target_bir_lowering=False, debug=False)

    # ---------------- DRAM I/O ----------------
    csA_d = nc.dram_tensor("csA", (n_pieces, 64, SUB), BF16, kind="ExternalInput")
    csB_d = nc.dram_tensor("csB", (n_pieces, 64, SUB), BF16, kind="ExternalInput")
    a0_d = nc.dram_tensor("a0", (2, 64, SUB), F32R, kind="ExternalInput")
    wf1_d = nc.dram_tensor("wf1", (128, 128), F32R, kind="ExternalInput")
    wf2_d = nc.dram_tensor("wf2", (128, 128), BF16, kind="ExternalInput")
    wf3_d = nc.dram_tensor("wf3", (128, 256), BF16, kind="ExternalInput")
    w0_d = nc.dram_tensor("w0", (128, 7 * 128), BF16, kind="ExternalInput")
    sdr_d = nc.dram_tensor("sdr", (128, 2, 32), FP8, kind="ExternalInput")
    wi1_d = nc.dram_tensor("wi1", (128, 64), F32R, kind="ExternalInput")
    wi2_d = nc.dram_tensor("wi2", (128, 32), F32R, kind="ExternalInput")
    fb1_d = nc.dram_tensor("fb1", (128, 1), F32, kind="ExternalInput")
    fb2_d = nc.dram_tensor("fb2", (128, 1), F32, kind="ExternalInput")
    fb3_d = nc.dram_tensor("fb3", (128, 2), F32, kind="ExternalInput")
    ib1_d = nc.dram_tensor("ib1", (64, 1), F32, kind="ExternalInput")
    ib2_d = nc.dram_tensor("ib2", (128, 1), F32, kind="ExternalInput")
    out_d = nc.dram_tensor("out", (2, n_knots, 64, SUB), F32R, kind="ExternalOutput")

    sched = _schedule(num_steps)

    with tile.TileContext(nc) as tc:
        with (
            tc.tile_pool(name="wpool", bufs=1) as wpool,
            tc.tile_pool(name="cs", bufs=3) as cspool,
            tc.tile_pool(name="zp", bufs=2) as zpool,
            tc.tile_pool(name="hp", bufs=2) as hpool,
            tc.tile_pool(name="fp", bufs=2) as fpool,
            tc.tile_pool(name="pp", bufs=2) as ppool,
            tc.tile_pool(name="mlp_ps", bufs=1, space="PSUM") as mlp_ps,
            tc.tile_pool(name="rep_ps", bufs=1, space="PSUM") as rep_ps,
            tc.tile_pool(name="k_ps", bufs=1, space="PSUM") as k_ps,
        ):
            _tn = [0]

            def mktile(pool, shape, tag, dt=F32, bufs=None):
                _tn[0] += 1
                return pool.tile(list(shape), dt, tag=tag,
                                 name=f"{tag}_{_tn[0]}", bufs=bufs)

            def mm(out, lhsT, rhs, **kw):
                if lhsT.dtype == F32:
                    lhsT = lhsT.bitcast(F32R)
                if rhs.dtype == F32:
                    rhs = rhs.bitcast(F32R)
                nc.tensor.matmul(out, lhsT, rhs, **kw)

            # ---------------- load weights ----------------
            def wtile(dram, shape, dt=F32):
                t = mktile(wpool, shape, dram.name + "_t", dt=dt)
                nc.sync.dma_start(t[:], dram.ap())
                return t

            wf1 = wtile(wf1_d, (128, 128), dt=F32R)
            wf2 = wtile(wf2_d, (128, 128), dt=BF16)
            wf3 = wtile(wf3_d, (128, 256), dt=BF16)
            w0 = wtile(w0_d, (128, 7 * 128), dt=BF16)
            sdr = wtile(sdr_d, (128, 2, 32), dt=FP8)
            wi1 = wtile(wi1_d, (128, 64), dt=F32R)
            wi2 = wtile(wi2_d, (128, 32), dt=F32R)
            fb1 = wtile(fb1_d, (128, 1))
            fb2 = wtile(fb2_d, (128, 1))
            fb3 = wtile(fb3_d, (128, 2))
            ib1 = wtile(ib1_d, (64, 1))
            ib2 = wtile(ib2_d, (128, 1))
            a0 = [None, None]
            for g in range(2):
                a0[g] = mktile(wpool, [64, SUB], f"a0_{g}", dt=F32R)
                nc.sync.dma_start(a0[g][:], a0_d.ap()[g])

            # ---------------- coefficient slab prefetch ----------------
            cs_dram = [csA_d, csB_d]
            cs_tiles = [{}, {}]

            def load_piece(p):
                if p >= n_pieces:
                    return
                for g in range(2):
                    t = mktile(cspool, [64, SUB], f"cs{g}", dt=BF16)
                    nc.sync.dma_start(t[:], cs_dram[g].ap()[p])
                    cs_tiles[g][p] = t

            for p in range(min(3, n_pieces)):
                load_piece(p)

            def readout(g, ztile, l):
                nc.sync.dma_start(out_d.ap()[g, l], ztile[:])

            # ---------------- per-group work streams ----------------
            STT = nc.vector.scalar_tensor_tensor
            # SBUF-only z-combination ops can run on the idle GpSimd engine
            STT2 = nc.gpsimd.scalar_tensor_tensor if GPS else STT
            MUL2 = nc.gpsimd.tensor_scalar_mul if GPS else nc.vector.tensor_scalar_mul

            def group_stream(g):
                # ---- z0 init ----
                zg = mktile(zpool, [64, SUB], f"z{g}", dt=F32R, bufs=3)
                for q in range(2):
                    h0ps = mktile(rep_ps, [128, GN], "rep")
                    mm(h0ps[0:64, 0:SUB], wi1[32 * q:32 * q + 8, 0:64],
                       a0[g][32 * q:32 * q + 8, :], tile_position=(32 * q, 0))
                    h0 = mktile(hpool, [64, SUB], "h0", dt=F32R)
                    nc.scalar.activation(h0[:], h0ps[0:64, 0:SUB], AF.Relu,
                                         bias=ib1[:])
                    zi_ps = mktile(k_ps, [32, GN], "kacc")
                    mm(zi_ps[:, 0:SUB], wi2[0:64, 0:32], h0[:])
                    nc.scalar.activation(zg[32 * q:32 * q + 32, :],
                                         zi_ps[:, 0:SUB],
                                         AF.Identity, bias=ib2[0:32, :])
                z = zg
                yield
                readout(g, z, 0)
                yield
                for s in range(num_steps):
                    if g == 0 and s % 2 == 0:
                        load_piece(s // 2 + 3)
                    zs = [None, z, None, None, None]
                    for e in range(4):
                        piece, fi = sched[s][e]
                        z_in = zs[e + 1]
                        cs = cs_tiles[g][piece]
                        # S1: mm1
                        h1ps = mktile(mlp_ps, [128, GN], f"mlp{g}")
                        for q in range(2):
                            mm(h1ps[:, q * SUB:(q + 1) * SUB],
                               wf1[32 * q:32 * q + 32, :],
                               z_in[32 * q:32 * q + 32, :],
                               tile_position=(32 * q, 0))
                        yield
                        # S2: tanh h1
                        h1 = mktile(hpool, [128, GN], f"h1_{g}", dt=BF16)
                        nc.scalar.activation(h1[:], h1ps[:], AF.Tanh, bias=fb1[:])
                        yield
                        # S3: mm2
                        h2ps = mktile(mlp_ps, [128, GN], f"mlp{g}")
                        for n2 in range(2):
                            mm(h2ps[:, n2 * 512:(n2 + 1) * 512], wf2[:, :],
                               h1[:, n2 * 512:(n2 + 1) * 512])
                        yield
                        # S4: tanh h2
                        h2 = mktile(hpool, [128, GN], f"h2_{g}", dt=BF16)
                        nc.scalar.activation(h2[:], h2ps[:], AF.Tanh, bias=fb2[:])
                        yield
                        # S5: mm3 t0
                        P = mktile(ppool, [128, 2, GN], f"P_{g}", dt=FP8)
                        f3p0 = mktile(mlp_ps, [128, GN], f"mlp{g}")
                        for n2 in range(2):
                            mm(f3p0[:, n2 * 512:(n2 + 1) * 512], wf3[:, 0:128],
                               h2[:, n2 * 512:(n2 + 1) * 512])
                        yield
                        # S6: tanh F0
                        F0 = mktile(fpool, [128, GN], f"F_{g}", dt=BF16)
                        nc.scalar.activation(F0[:], f3p0[:], AF.Tanh,
                                             bias=fb3[:, 0:1])
                        yield
                        # S7: mm3 t1
                        f3p1 = mktile(mlp_ps, [128, GN], f"mlp{g}")
                        for n2 in range(2):
                            mm(f3p1[:, n2 * 512:(n2 + 1) * 512], wf3[:, 128:256],
                               h2[:, n2 * 512:(n2 + 1) * 512])
                        yield
                        # S8: rep (q0+q1) on PE, then P plane0 on DVE
                        rep = mktile(rep_ps, [128, GN], "rep")
                        for q in range(2):
                            mm(rep[:, q * SUB:(q + 1) * SUB],
                               w0[32 * q:32 * q + 24, fi * 128:(fi + 1) * 128],
                               cs[32 * q:32 * q + 24, :],
                               tile_position=(32 * q, 0))
                        nc.vector.tensor_tensor(
                            P[:, 0, :], F0[:, :], rep[:, :], OP.mult)
                        yield
                        # S9: tanh F1
                        F1 = mktile(fpool, [128, GN], f"F_{g}", dt=BF16)
                        nc.scalar.activation(F1[:], f3p1[:], AF.Tanh,
                                             bias=fb3[:, 1:2])
                        yield
                        # S10: P plane1 + kacc q0 (fp8 DoubleRow)
                        nc.vector.tensor_tensor(
                            P[:, 1, :], F1[:, :], rep[:, :], OP.mult)
                        kacc = mktile(k_ps, [32, GN], "kacc")

                        def ksum(q):
                            qs = slice(q * SUB, (q + 1) * SUB)
                            if KSUM_MODE == "dr":
                                nc.tensor.matmul(
                                    kacc[:, qs], sdr[:, :, :],
                                    P[:, :, qs], perf_mode=DR)
                            else:  # fp8acc: plain fp8, 2-pass accumulate
                                for i in range(2):
                                    nc.tensor.matmul(
                                        kacc[:, qs], sdr[:, i, :],
                                        P[:, i, qs],
                                        start=(i == 0), stop=(i == 1))

                        ksum(0)
                        yield
                        # S11: kacc q1
                        ksum(1)
                        yield

                        # S12: z update; k STTs (psum) on DVE, SBUF-only ops
                        # optionally on GpSimd.
                        def kstt(dst, scal, other_tile, cast=False):
                            for q in range(2):
                                o = other_tile[32 * q:32 * q + 32, :]
                                if cast:
                                    o = o.bitcast(F32)
                                STT(dst[32 * q:32 * q + 32, :],
                                    kacc[:, q * SUB:(q + 1) * SUB], scal, o,
                                    OP.mult, OP.add)

                        if e == 0:
                            z2 = mktile(zpool, [64, SUB], f"z2_{g}", dt=F32R)
                            kstt(z2, DT / 3.0, z, cast=True)
                            zs[2] = z2
                        elif e == 1:
                            tmp = mktile(zpool, [64, SUB], f"tmp_{g}")
                            STT2(tmp[:], z[:].bitcast(F32), 2.0,
                                 zs[2][:].bitcast(F32), OP.mult, OP.subtract)
                            z3 = mktile(zpool, [64, SUB], f"z3_{g}", dt=F32R)
                            kstt(z3, DT, tmp)
                            zs[3] = z3
                        elif e == 2:
                            tmp2 = mktile(zpool, [64, SUB], f"tmp2_{g}")
                            STT2(tmp2[:], zs[2][:].bitcast(F32), 2.0,
                                 zs[3][:].bitcast(F32), OP.mult, OP.subtract)
                            z4 = mktile(zpool, [64, SUB], f"z4_{g}", dt=F32R)
                            kstt(z4, DT, tmp2)
                            zs[4] = z4
                        else:
                            t3 = mktile(zpool, [64, SUB], f"t3_{g}")
                            STT2(t3[:], zs[3][:].bitcast(F32), 6.0,
                                 z[:].bitcast(F32), OP.mult, OP.subtract)
                            t4 = mktile(zpool, [64, SUB], f"t4_{g}")
                            STT2(t4[:], zs[4][:].bitcast(F32), 3.0, t3[:],
                                 OP.mult, OP.add)
                            u = mktile(zpool, [64, SUB], f"u_{g}")
                            kstt(u, DT, t4)
                            znew = mktile(zpool, [64, SUB], f"z{g}", dt=F32R,
                                          bufs=3)
                            MUL2(znew[:], u[:], 0.125)
                            z = znew
                        yield
                    if s % 2 == 1:
                        l = (s + 1) // 2
                        if l < n_knots:
                            readout(g, z, l)
                            yield

            # anti-phase scheduling
            OFFSET = 6
            streams = [group_stream(0), group_stream(1)]
            for _ in range(OFFSET):
                next(streams[0])
            alive = [True, True]
            while alive[0] or alive[1]:
                for gi in (1, 0):
                    if alive[gi]:
                        try:
                            next(streams[gi])
                        except StopIteration:
                            alive[gi] = False

    t_trace = time.time()
    nc.compile()
    t_compile = time.time()
    print(f"[kernel] trace {t_trace - t_start:.1f}s, "
          f"tile-schedule+compile {t_compile - t_trace:.1f}s, "
          f"instructions: {sum(len(b.instructions) for f in nc.m.functions for b in f.blocks)}")
    _BUILD_CACHE[key] = nc
    return nc


# =====================================================================
# host-side data prep
# =====================================================================

def _prep_weights(iW1, ib1, iW2, ib2, fW1, fb1, fW2, fb2, fW3, fb3):
    import ml_dtypes
    R = np.zeros((C, 128), np.float32)
    for j in range(128):
        R[j % C, j] = 1.0
    w0 = np.zeros((128, 7 * 128), np.float32)
    for fi in range(7):
        f = fi / 6.0
        blk = np.concatenate([R, f * R, f * f * R,
                              np.zeros((8, 128), np.float32)], axis=0)
        w0[:, fi * 128:(fi + 1) * 128] = np.tile(blk, (4, 1))
    sdr = np.zeros((128, 2, 32), np.float32)
    for j in range(128):
        sdr[j, 0, j // C] = 1.0       # plane0: F0 row j -> h = j//8 (0..15)
        sdr[j, 1, 16 + j // C] = 1.0  # plane1: F1 row j -> h = 16 + j//8
    fb3v = np.asarray(fb3, np.float32).reshape(256)
    d = {
        "wf1": np.tile(fW1, (4, 1)),
        "wf2": fW2,
        "wf3": fW3,
        "w0": w0,
        "sdr": sdr,
        "wi1": np.tile(np.concatenate([iW1, np.zeros((24, 64), np.float32)], 0), (4, 1)),
        "wi2": np.tile(iW2, (2, 1)),
        "fb1": fb1.reshape(128, 1),
        "fb2": fb2.reshape(128, 1),
        "fb3": np.stack([fb3v[0:128], fb3v[128:256]], axis=1).copy(),
        "ib1": ib1.reshape(64, 1),
        "ib2": np.tile(ib2.reshape(32, 1), (4, 1)),
    }
    out = {k: np.ascontiguousarray(v, dtype=np.float32) for k, v in d.items()}
    for k in ("wf2", "wf3", "w0"):
        out[k] = out[k].astype(ml_dtypes.bfloat16)
    out["sdr"] = out["sdr"].astype(ml_dtypes.float8_e4m3fn)
    return out


def _prep_coeffs(coeffs, n_pieces):
    """coeffs (B, NP, 32) -> per-core csA/csB (n_pieces, 64, 512), a0 (2,64,512)."""
    import ml_dtypes
    npc = coeffs.shape[1]
    x = np.asarray(coeffs, np.float32).reshape(CORES, 2, 2, SUB, npc, 32)
    sl = x[..., 8:32]                                  # (8, 2, 2, 512, np, 24)
    sl = np.transpose(sl, (0, 1, 4, 2, 5, 3))          # (8, 2, np, 2, 24, 512)
    sl = np.pad(sl, ((0, 0),) * 4 + ((0, 8), (0, 0)))  # (8, 2, np, 2, 32, 512)
    sl = sl.reshape(CORES, 2, npc, 64, SUB)[:, :, :n_pieces]
    sl = np.ascontiguousarray(sl).astype(ml_dtypes.bfloat16)
    a = x[:, :, :, :, 0, 0:8]                          # (8, 2, 2, 512, 8)
    a = np.transpose(a, (0, 1, 2, 4, 3))               # (8, 2, 2, 8, 512)
    a = np.pad(a, ((0, 0),) * 3 + ((0, 24), (0, 0)))   # (8, 2, 2, 32, 512)
    a = np.ascontiguousarray(a.reshape(CORES, 2, 64, SUB), dtype=np.float32)
    return sl, a


def _unscramble_out(res_list, rW1, rb1, rW2, rb2, n_knots=L):
    """res_list: per-core dicts with 'out' (2, n_knots, 64, 512) raw z states."""
    zs = []
    for c in range(CORES):
        o = res_list[c]["out"]                   # (2, L, 64, 512): g, l, 32q+h, n
        o = o.reshape(2, n_knots, 2, 32, SUB)    # g, l, q, h, n
        o = np.transpose(o, (0, 2, 4, 1, 3))     # g, q, n, l, h
        zs.append(o.reshape(BC, n_knots, 32))
    z_eval = np.concatenate(zs, axis=0)          # (B, L, 32)
    r1 = np.maximum(z_eval @ np.asarray(rW1, np.float32) +
                    np.asarray(rb1, np.float32), 0.0)
    out = r1 @ np.asarray(rW2, np.float32) + np.asarray(rb2, np.float32)
    return np.ascontiguousarray(out, dtype=np.float32)


LAST_RES = None


def kernel(coeffs, t_eval, iW1, ib1, iW2, ib2, fW1, fb1, fW2, fb2, fW3, fb3,
           rW1, rb1, rW2, rb2, _num_steps=STEPS, _n_pieces=NPIECE, _n_knots=L,
           _time_iters=0, _trace=False, _tmpdir=None):
    global LAST_RES
    nc = _build(_num_steps, _n_pieces, _n_knots)
    w = _prep_weights(iW1, ib1, iW2, ib2, fW1, fb1, fW2, fb2, fW3, fb3)
    sl, a0 = _prep_coeffs(coeffs, _n_pieces)
    in_maps = []
    for c in range(CORES):
        m = dict(w)
        m["csA"] = sl[c, 0]
        m["csB"] = sl[c, 1]
        m["a0"] = a0[c]
        in_maps.append(m)
    res = bass_utils.run_bass_kernel_spmd(
        nc, in_maps, core_ids=list(range(CORES)),
        trace=_trace, tmpdir=_tmpdir)
    LAST_RES = res
    return _unscramble_out(res.results, rW1, rb1, rW2, rb2, _n_knots)


# revision 19
# speedup vs baseline: 1.0288x; 1.0288x over previous
"""NeuralCDE Bass kernel for Trainium2 (8 NeuronCores, data parallel).

Computes the reference NeuralCDE: cubic-spline-controlled ODE integrated with
torchdiffeq-style rk4 (3/8 rule) over 254 fixed steps, MLP vector field
(32 -> 128 -> 128 -> 32x8 with tanh), initial MLP on device, readout MLP on
host.

Layout (per core, batch BC=2048, two anti-phased groups of GN=1024):
  - batch split into 2 groups of 1024, each 2 subchunks of 512; the two
    groups' pipelines are emitted anti-phased (half-eval stage offset) so
    one group's PE matmuls overlap the other group's ACT/DVE stages.
  - activations are feature-major: z packed as (64, 512) tiles with row
    32*q + h (q = local subchunk, h = hidden dim), batch along free dim.
  - MLP: mm1 fp32r (K=32 per q via tile_position rows 0/32 -- rows 64/96
    crash the device, so subchunk count stays at 2), mm2/mm3 bf16, tanh
    on ACT with fused bias (bf16 out).
  - spline derivative: rep = [R; f R; f^2 R]^T @ coeff-slab on PE into a
    single (128,1024) psum tile (R replicates channel c to rows j%8==c).
  - P = tanh(F) * rep computed by DVE into an fp8e4 (128, 2, 1024) tile
    (plane i = F_i row block); channel sum k = S^T P via one fp8
    DoubleRow matmul per subchunk (K=2x128 planes in a single pass) into
    kacc (32, 1024) -- 2-4x cheaper on PE than the bf16 2-pass sum.
  - rk4 z-updates via scalar_tensor_tensor identities consuming each k_i
    immediately:
       z2 = z + (dt/3) k1
       z3 = dt*k2 + (2z - z2)
       z4 = dt*k3 + (2*z2 - z3)
       z5 = ((dt*k4 + (3*z4 + (6*z3 - z)))) / 8
    k-consuming STTs read psum and stay on DVE; the SBUF-only ops
    (tmp/t3/t4/znew) can be offloaded to the idle GpSimd engine
    (NCDE_GPS=1) to unload DVE.
"""

import os
import sys
import time

sys.path.insert(0, "/opt/trn_rl_repo")

import numpy as np

import concourse.bacc as bacc
import concourse.bass as bass
from concourse import bass_utils, mybir, tile

F32 = mybir.dt.float32
F32R = mybir.dt.float32r
BF16 = mybir.dt.bfloat16
FP8 = mybir.dt.float8e4
AF = mybir.ActivationFunctionType
OP = mybir.AluOpType
DR = mybir.MatmulPerfMode.DoubleRow

KSUM_MODE = os.environ.get("NCDE_KSUM", "dr")   # dr | fp8acc
GPS = bool(int(os.environ.get("NCDE_GPS", "0")))
# HAM keep-warm: emit N dependency-free ldweights on PE in matmul-less
# stages so the activity monitor sees a busy PE and un-gates the 2.4 GHz
# clock (otherwise the PE idles in bursts and is clamped to 1.2 GHz).
LDW = int(os.environ.get("NCDE_LDW", "0"))

CORES = 8
B = 16384
BC = B // CORES          # 2048 batch per core
SUB = 512                # subchunk batch
GN = 2 * SUB             # 1024 batch per group
L = 128                  # knots
NPIECE = L - 1           # 127
STEPS = 2 * (L - 1)      # 254
HID = 32
C = 8
DT = 0.5

_BUILD_CACHE = {}


def _schedule(num_steps):
    """Per (step, eval) -> (piece, frac_index); frac = fi/6."""
    sched = []
    for s in range(num_steps):
        evs = []
        for e in range(4):
            T = 3 * s + e
            idx = min(T // 6, NPIECE - 1)
            fi = T - 6 * idx
            evs.append((idx, fi))
        sched.append(evs)
    return sched


def _build(num_steps=STEPS, n_pieces=NPIECE, n_knots=L):
    key = (num_steps, n_pieces, n_knots)
    if key in _BUILD_CACHE:
        return _BUILD_CACHE[key]

    t_start = time.time()
    nc = bacc.Bacc("TRN2", target_bir_lowering=False, debug=False)

    # ---------------- DRAM I/O ----------------
    csA_d = nc.dram_tensor("csA", (n_pieces, 64, SUB), BF16, kind="ExternalInput")
    csB_d = nc.dram_tensor("csB", (n_pieces, 64, SUB), BF16, kind="ExternalInput")
    a0_d = nc.dram_tensor("a0", (2, 64, SUB), F32R, kind="ExternalInput")
    wf1_d = nc.dram_tensor("wf1", (128, 128), F32R, kind="ExternalInput")
    wf2_d = nc.dram_tensor("wf2", (128, 128), BF16, kind="ExternalInput")
    wf3_d = nc.dram_tensor("wf3", (128, 256), BF16, kind="ExternalInput")
    w0_d = nc.dram_tensor("w0", (128, 7 * 128), BF16, kind="ExternalInput")
    sdr_d = nc.dram_tensor("sdr", (128, 2, 32), FP8, kind="ExternalInput")
    wi1_d = nc.dram_tensor("wi1", (128, 64), F32R, kind="ExternalInput")
    wi2_d = nc.dram_tensor("wi2", (128, 32), F32R, kind="ExternalInput")
    fb1_d = nc.dram_tensor("fb1", (128, 1), F32, kind="ExternalInput")
    fb2_d = nc.dram_tensor("fb2", (128, 1), F32, kind="ExternalInput")
    fb3_d = nc.dram_tensor("fb3", (128, 2), F32, kind="ExternalInput")
    ib1_d = nc.dram_tensor("ib1", (64, 1), F32, kind="ExternalInput")
    ib2_d = nc.dram_tensor("ib2", (128, 1), F32, kind="ExternalInput")
    out_d = nc.dram_tensor("out", (2, n_knots, 64, SUB), F32R, kind="ExternalOutput")

    sched = _schedule(num_steps)

    with tile.TileContext(nc) as tc:
        with (
            tc.tile_pool(name="wpool", bufs=1) as wpool,
            tc.tile_pool(name="cs", bufs=3) as cspool,
            tc.tile_pool(name="zp", bufs=2) as zpool,
            tc.tile_pool(name="hp", bufs=2) as hpool,
            tc.tile_pool(name="fp", bufs=2) as fpool,
            tc.tile_pool(name="pp", bufs=2) as ppool,
            tc.tile_pool(name="mlp_ps", bufs=1, space="PSUM") as mlp_ps,
            tc.tile_pool(name="rep_ps", bufs=2, space="PSUM") as rep_ps,
            tc.tile_pool(name="k_ps", bufs=2, space="PSUM") as k_ps,
        ):
            _tn = [0]

            def mktile(pool, shape, tag, dt=F32, bufs=None):
                _tn[0] += 1
                return pool.tile(list(shape), dt, tag=tag,
                                 name=f"{tag}_{_tn[0]}", bufs=bufs)

            def mm(out, lhsT, rhs, **kw):
                if lhsT.dtype == F32:
                    lhsT = lhsT.bitcast(F32R)
                if rhs.dtype == F32:
                    rhs = rhs.bitcast(F32R)
                nc.tensor.matmul(out, lhsT, rhs, **kw)

            # ---------------- load weights ----------------
            def wtile(dram, shape, dt=F32):
                t = mktile(wpool, shape, dram.name + "_t", dt=dt)
                nc.sync.dma_start(t[:], dram.ap())
                return t

            wf1 = wtile(wf1_d, (128, 128), dt=F32R)
            wf2 = wtile(wf2_d, (128, 128), dt=BF16)
            wf3 = wtile(wf3_d, (128, 256), dt=BF16)
            w0 = wtile(w0_d, (128, 7 * 128), dt=BF16)
            sdr = wtile(sdr_d, (128, 2, 32), dt=FP8)
            wi1 = wtile(wi1_d, (128, 64), dt=F32R)
            wi2 = wtile(wi2_d, (128, 32), dt=F32R)
            fb1 = wtile(fb1_d, (128, 1))
            fb2 = wtile(fb2_d, (128, 1))
            fb3 = wtile(fb3_d, (128, 2))
            ib1 = wtile(ib1_d, (64, 1))
            ib2 = wtile(ib2_d, (128, 1))
            a0 = [None, None]
            for g in range(2):
                a0[g] = mktile(wpool, [64, SUB], f"a0_{g}", dt=F32R)
                nc.sync.dma_start(a0[g][:], a0_d.ap()[g])

            # ---------------- coefficient slab prefetch ----------------
            cs_dram = [csA_d, csB_d]
            cs_tiles = [{}, {}]

            def load_piece(p):
                if p >= n_pieces:
                    return
                for g in range(2):
                    t = mktile(cspool, [64, SUB], f"cs{g}", dt=BF16)
                    nc.sync.dma_start(t[:], cs_dram[g].ap()[p])
                    cs_tiles[g][p] = t

            for p in range(min(3, n_pieces)):
                load_piece(p)

            def readout(g, ztile, l):
                nc.sync.dma_start(out_d.ap()[g, l], ztile[:])

            # ---------------- per-group work streams ----------------
            STT = nc.vector.scalar_tensor_tensor
            # SBUF-only z-combination ops can run on the idle GpSimd engine
            STT2 = nc.gpsimd.scalar_tensor_tensor if GPS else STT
            MUL2 = nc.gpsimd.tensor_scalar_mul if GPS else nc.vector.tensor_scalar_mul

            def warm():
                for _ in range(LDW):
                    nc.tensor.ldweights(wf2[:, :])

            def group_stream(g):
                # ---- z0 init ----
                zg = mktile(zpool, [64, SUB], f"z{g}", dt=F32R, bufs=3)
                for q in range(2):
                    h0ps = mktile(rep_ps, [128, SUB], "rep")
                    mm(h0ps[0:64, :], wi1[32 * q:32 * q + 8, 0:64],
                       a0[g][32 * q:32 * q + 8, :], tile_position=(32 * q, 0))
                    h0 = mktile(hpool, [64, SUB], "h0", dt=F32R)
                    nc.scalar.activation(h0[:], h0ps[0:64, :], AF.Relu,
                                         bias=ib1[:])
                    zi_ps = mktile(k_ps, [32, SUB], "kacc")
                    mm(zi_ps[:], wi2[0:64, 0:32], h0[:])
                    nc.scalar.activation(zg[32 * q:32 * q + 32, :],
                                         zi_ps[:],
                                         AF.Identity, bias=ib2[0:32, :])
                z = zg
                yield
                readout(g, z, 0)
                yield
                for s in range(num_steps):
                    if g == 0 and s % 2 == 0:
                        load_piece(s // 2 + 3)
                    zs = [None, z, None, None, None]
                    for e in range(4):
                        piece, fi = sched[s][e]
                        z_in = zs[e + 1]
                        cs = cs_tiles[g][piece]
                        # S1: mm1
                        h1ps = mktile(mlp_ps, [128, GN], f"mlp{g}")
                        for q in range(2):
                            mm(h1ps[:, q * SUB:(q + 1) * SUB],
                               wf1[32 * q:32 * q + 32, :],
                               z_in[32 * q:32 * q + 32, :],
                               tile_position=(32 * q, 0))
                        yield
                        # S2: tanh h1
                        h1 = mktile(hpool, [128, GN], f"h1_{g}", dt=BF16)
                        nc.scalar.activation(h1[:], h1ps[:], AF.Tanh, bias=fb1[:])
                        yield
                        # S3: mm2
                        h2ps = mktile(mlp_ps, [128, GN], f"mlp{g}")
                        for n2 in range(2):
                            mm(h2ps[:, n2 * 512:(n2 + 1) * 512], wf2[:, :],
                               h1[:, n2 * 512:(n2 + 1) * 512])
                        yield
                        # S4: tanh h2
                        h2 = mktile(hpool, [128, GN], f"h2_{g}", dt=BF16)
                        nc.scalar.activation(h2[:], h2ps[:], AF.Tanh, bias=fb2[:])
                        yield
                        # S5: mm3 t0
                        P = mktile(ppool, [128, 2, GN], f"P_{g}", dt=FP8)
                        f3p0 = mktile(mlp_ps, [128, GN], f"mlp{g}")
                        for n2 in range(2):
                            mm(f3p0[:, n2 * 512:(n2 + 1) * 512], wf3[:, 0:128],
                               h2[:, n2 * 512:(n2 + 1) * 512])
                        yield
                        # S6: tanh F0
                        F0 = mktile(fpool, [128, GN], f"F_{g}", dt=BF16)
                        nc.scalar.activation(F0[:], f3p0[:], AF.Tanh,
                                             bias=fb3[:, 0:1])
                        yield
                        # S7: mm3 t1
                        f3p1 = mktile(mlp_ps, [128, GN], f"mlp{g}")
                        for n2 in range(2):
                            mm(f3p1[:, n2 * 512:(n2 + 1) * 512], wf3[:, 128:256],
                               h2[:, n2 * 512:(n2 + 1) * 512])
                        yield
                        # S8: rep (q0,q1) on PE, then P plane0 on DVE
                        reps = []
                        for q in range(2):
                            rq = mktile(rep_ps, [128, SUB], "rep")
                            mm(rq[:, :],
                               w0[32 * q:32 * q + 24, fi * 128:(fi + 1) * 128],
                               cs[32 * q:32 * q + 24, :],
                               tile_position=(32 * q, 0))
                            reps.append(rq)
                        for q in range(2):
                            nc.vector.tensor_tensor(
                                P[:, 0, q * SUB:(q + 1) * SUB],
                                F0[:, q * SUB:(q + 1) * SUB],
                                reps[q][:, :], OP.mult)
                        yield
                        # S9: tanh F1 + P plane1 q0
                        F1 = mktile(fpool, [128, GN], f"F_{g}", dt=BF16)
                        nc.scalar.activation(F1[:], f3p1[:], AF.Tanh,
                                             bias=fb3[:, 1:2])
                        nc.vector.tensor_tensor(
                            P[:, 1, 0:SUB], F1[:, 0:SUB],
                            reps[0][:, :], OP.mult)
                        yield
                        # S10: P plane1 q1 + kacc q0 (fp8 DoubleRow)
                        kq = []

                        def ksum(q, kt):
                            qs = slice(q * SUB, (q + 1) * SUB)
                            if KSUM_MODE == "dr":
                                nc.tensor.matmul(
                                    kt[:], sdr[:, :, :],
                                    P[:, :, qs], perf_mode=DR)
                            else:  # fp8acc: plain fp8, 2-pass accumulate
                                for i in range(2):
                                    nc.tensor.matmul(
                                        kt[:], sdr[:, i, :],
                                        P[:, i, qs],
                                        start=(i == 0), stop=(i == 1))

                        nc.vector.tensor_tensor(
                            P[:, 1, SUB:GN], F1[:, SUB:GN],
                            reps[1][:, :], OP.mult)
                        kt = mktile(k_ps, [32, SUB], "kacc")
                        ksum(0, kt)
                        kq.append(kt)
                        yield
                        # S11: kacc q1
                        kt = mktile(k_ps, [32, SUB], "kacc")
                        ksum(1, kt)
                        kq.append(kt)
                        yield

                        # S12: z update; k STTs (psum) on DVE, SBUF-only ops
                        # optionally on GpSimd.
                        def kstt(dst, scal, other_tile, cast=False):
                            for q in range(2):
                                o = other_tile[32 * q:32 * q + 32, :]
                                if cast:
                                    o = o.bitcast(F32)
                                STT(dst[32 * q:32 * q + 32, :],
                                    kq[q][:], scal, o,
                                    OP.mult, OP.add)

                        if e == 0:
                            z2 = mktile(zpool, [64, SUB], f"z2_{g}", dt=F32R)
                            kstt(z2, DT / 3.0, z, cast=True)
                            zs[2] = z2
                        elif e == 1:
                            tmp = mktile(zpool, [64, SUB], f"tmp_{g}")
                            STT2(tmp[:], z[:].bitcast(F32), 2.0,
                                 zs[2][:].bitcast(F32), OP.mult, OP.subtract)
                            z3 = mktile(zpool, [64, SUB], f"z3_{g}", dt=F32R)
                            kstt(z3, DT, tmp)
                            zs[3] = z3
                        elif e == 2:
                            tmp2 = mktile(zpool, [64, SUB], f"tmp2_{g}")
                            STT2(tmp2[:], zs[2][:].bitcast(F32), 2.0,
                                 zs[3][:].bitcast(F32), OP.mult, OP.subtract)
                            z4 = mktile(zpool, [64, SUB], f"z4_{g}", dt=F32R)
                            kstt(z4, DT, tmp2)
                            zs[4] = z4
                        else:
                            t3 = mktile(zpool, [64, SUB], f"t3_{g}")
                            STT2(t3[:], zs[3][:].bitcast(F32), 6.0,
                                 z[:].bitcast(F32), OP.mult, OP.subtract)
                            t4 = mktile(zpool, [64, SUB], f"t4_{g}")
                            STT2(t4[:], zs[4][:].bitcast(F32), 3.0, t3[:],
                                 OP.mult, OP.add)
                            u = mktile(zpool, [64, SUB], f"u_{g}")
                            kstt(u, DT, t4)
                            znew = mktile(zpool, [64, SUB], f"z{g}", dt=F32R,
                                          bufs=3)
                            MUL2(znew[:], u[:], 0.125)
                            z = znew
                        yield
                    if s % 2 == 1:
                        l = (s + 1) // 2
                        if l < n_knots:
                            readout(g, z, l)
                            yield

            # anti-phase scheduling
            OFFSET = 6
            streams = [group_stream(0), group_stream(1)]
            for _ in range(OFFSET):
                next(streams[0])
            alive = [True, True]
            while alive[0] or alive[1]:
                for gi in (1, 0):
                    if alive[gi]:
                        try:
                            next(streams[gi])
                        except StopIteration:
                            alive[gi] = False

    t_trace = time.time()
    nc.compile()
    t_compile = time.time()
    print(f"[kernel] trace {t_trace - t_start:.1f}s, "
          f"tile-schedule+compile {t_compile - t_trace:.1f}s, "
          f"instructions: {sum(len(b.instructions) for f in nc.m.functions for b in f.blocks)}")
    _BUILD_CACHE[key] = nc
    return nc


# =====================================================================
# host-side data prep
# =====================================================================

def _prep_weights(iW1, ib1, iW2, ib2, fW1, fb1, fW2, fb2, fW3, fb3):
    import ml_dtypes
    R = np.zeros((C, 128), np.float32)
    for j in range(128):
        R[j % C, j] = 1.0
    w0 = np.zeros((128, 7 * 128), np.float32)
    for fi in range(7):
        f = fi / 6.0
        blk = np.concatenate([R, f * R, f * f * R,
                              np.zeros((8, 128), np.float32)], axis=0)
        w0[:, fi * 128:(fi + 1) * 128] = np.tile(blk, (4, 1))
    sdr = np.zeros((128, 2, 32), np.float32)
    for j in range(128):
        sdr[j, 0, j // C] = 1.0       # plane0: F0 row j -> h = j//8 (0..15)
        sdr[j, 1, 16 + j // C] = 1.0  # plane1: F1 row j -> h = 16 + j//8
    fb3v = np.asarray(fb3, np.float32).reshape(256)
    d = {
        "wf1": np.tile(fW1, (4, 1)),
        "wf2": fW2,
        "wf3": fW3,
        "w0": w0,
        "sdr": sdr,
        "wi1": np.tile(np.concatenate([iW1, np.zeros((24, 64), np.float32)], 0), (4, 1)),
        "wi2": np.tile(iW2, (2, 1)),
        "fb1": fb1.reshape(128, 1),
        "fb2": fb2.reshape(128, 1),
        "fb3": np.stack([fb3v[0:128], fb3v[128:256]], axis=1).copy(),
        "ib1": ib1.reshape(64, 1),
        "ib2": np.tile(ib2.reshape(32, 1), (4, 1)),
    }
    out = {k: np.ascontiguousarray(v, dtype=np.float32) for k, v in d.items()}
    for k in ("wf2", "wf3", "w0"):
        out[k] = out[k].astype(ml_dtypes.bfloat16)
    out["sdr"] = out["sdr"].astype(ml_dtypes.float8_e4m3fn)
    return out


def _prep_coeffs(coeffs, n_pieces):
    """coeffs (B, NP, 32) -> per-core csA/csB (n_pieces, 64, 512), a0 (2,64,512)."""
    import ml_dtypes
    npc = coeffs.shape[1]
    x = np.asarray(coeffs, np.float32).reshape(CORES, 2, 2, SUB, npc, 32)
    sl = x[..., 8:32]                                  # (8, 2, 2, 512, np, 24)
    sl = np.transpose(sl, (0, 1, 4, 2, 5, 3))          # (8, 2, np, 2, 24, 512)
    sl = np.pad(sl, ((0, 0),) * 4 + ((0, 8), (0, 0)))  # (8, 2, np, 2, 32, 512)
    sl = sl.reshape(CORES, 2, npc, 64, SUB)[:, :, :n_pieces]
    sl = np.ascontiguousarray(sl).astype(ml_dtypes.bfloat16)
    a = x[:, :, :, :, 0, 0:8]                          # (8, 2, 2, 512, 8)
    a = np.transpose(a, (0, 1, 2, 4, 3))               # (8, 2, 2, 8, 512)
    a = np.pad(a, ((0, 0),) * 3 + ((0, 24), (0, 0)))   # (8, 2, 2, 32, 512)
    a = np.ascontiguousarray(a.reshape(CORES, 2, 64, SUB), dtype=np.float32)
    return sl, a


def _unscramble_out(res_list, rW1, rb1, rW2, rb2, n_knots=L):
    """res_list: per-core dicts with 'out' (2, n_knots, 64, 512) raw z states."""
    zs = []
    for c in range(CORES):
        o = res_list[c]["out"]                   # (2, L, 64, 512): g, l, 32q+h, n
        o = o.reshape(2, n_knots, 2, 32, SUB)    # g, l, q, h, n
        o = np.transpose(o, (0, 2, 4, 1, 3))     # g, q, n, l, h
        zs.append(o.reshape(BC, n_knots, 32))
    z_eval = np.concatenate(zs, axis=0)          # (B, L, 32)
    r1 = np.maximum(z_eval @ np.asarray(rW1, np.float32) +
                    np.asarray(rb1, np.float32), 0.0)
    out = r1 @ np.asarray(rW2, np.float32) + np.asarray(rb2, np.float32)
    return np.ascontiguousarray(out, dtype=np.float32)


LAST_RES = None


def kernel(coeffs, t_eval, iW1, ib1, iW2, ib2, fW1, fb1, fW2, fb2, fW3, fb3,
           rW1, rb1, rW2, rb2, _num_steps=STEPS, _n_pieces=NPIECE, _n_knots=L,
           _time_iters=0, _trace=False, _tmpdir=None):
    global LAST_RES
    nc = _build(_num_steps, _n_pieces, _n_knots)
    w = _prep_weights(iW1, ib1, iW2, ib2, fW1, fb1, fW2, fb2, fW3, fb3)
    sl, a0 = _prep_coeffs(coeffs, _n_pieces)
    in_maps = []
    for c in range(CORES):
        m = dict(w)
        m["csA"] = sl[c, 0]
        m["csB"] = sl[c, 1]
        m["a0"] = a0[c]
        in_maps.append(m)
    res = bass_utils.run_bass_kernel_spmd(
        nc, in_maps, core_ids=list(range(CORES)),
        trace=_trace, tmpdir=_tmpdir)
    LAST_RES = res
    return _unscramble_out(res.results, rW1, rb1, rW2, rb2, _n_knots)
